# revision 54
# baseline (speedup 1.0000x reference)
"""Trainium2 Bass kernel for nn_DotAttention (B=4, Tq=Tv=2048, D=1024, 16 heads).

Load-balanced sharding: core c owns head-pair p=c (heads 2c, 2c+1 =
att-dim cols [128c, 128c+128)) of EVERY batch. Each core processes 4
"units", one per batch, ordered by ASCENDING NJ_b = ceil(len_b/128) so
the longest unit runs last and its many exp-paced iterations absorb the
final-projection backlog. All cores share one SPMD program specialized
on the NJ multiset (here (4, 8, 12, 14)): identical structure,
different data. Per-core PE work is sum_b(NJ_b)-proportional —
perfectly balanced vs. the max_b(NJ) every core paid under
(batch, head-group) sharding.

Per unit u (batch b_u):
  A_u: q/k/v projections for the pair's 128 dims (f16 matmuls, f32 psum)
  B_u: masked-softmax attention, energy^T layout [Tv, 2 heads x 512 Tq],
       denominators via a ones-column in v. The ctx matmuls trail the
       energy/exp stream by 2 iterations so the PE never waits on a
       fresh exp. Normalization (reciprocal -> copy ctx to SBUF ->
       PE-broadcast of the reciprocal row -> in-place scale) is deferred
       into the next ib / A window, off the critical path.
  C_u: final projection partial (128-dim contraction), spliced one tile
       per ~2 exp-paced iterations into B windows and drained in the
       PE-bound A windows (ACT does those copies; DVE the B ones).

DMA: all bulk transfers are single contiguous blocks (host pre-packs x
inputs span-major, weights partition-major, y partials tile-major) —
scattered 1KB-line patterns measured ~2x slower on hardware. Input
streams ride sync+gpsimd queues; scalar carries only activations so
dma_start issuance never blocks the exp stream.

Host: per batch, sum the 8 cores' f16 partials in f32, untile, and add
the constant (bv @ Wf + bf, exact because attention weights sum to 1).

All matmul operands are float16 (full PE rate at any tile size, half the
HBM traffic of f32); PSUM accumulation is f32 throughout, so accuracy
is comparable to the float32r baseline (~9e-4 rel err vs 2e-2 budget).
PSUM banks: pctx(2) + pcy(2) global; per-unit windows alternate
ppq(2)+ppv(2) [A] and e(2x2) [B] in the remaining 4 banks.
"""

import sys

sys.path.insert(0, "/opt/trn_rl_repo")

import numpy as np

import concourse.bacc as bacc
import concourse.tile as tile
import concourse.mybir as mybir
from concourse.bass_utils import run_bass_kernel_spmd

F32 = mybir.dt.float32
F16 = mybir.dt.float16
AF = mybir.ActivationFunctionType

B, T, D, ATT = 4, 2048, 1024, 1024
NH, DH = 16, 64
NCORES = 8
LARGE = 1e30
SW = 512  # time-span width per streamed input chunk

_cache = {}
REGIONS = []  # (label, first_instruction_id) marks for trace attribution


def _mark(nc, label):
    nid = nc.next_id() if callable(nc.next_id) else nc.next_id
    REGIONS.append((label, int(nid)))


def build_nc(njs, loop_n=1, parts="ABC"):
    """njs: tuple of per-unit NJ (Tv chunks of 128), descending."""
    njs = tuple(int(n) for n in njs)
    key = (njs, loop_n, parts)
    if key in _cache:
        return _cache[key]
    U = len(njs)
    nsvs = [(nj * 128 + SW - 1) // SW for nj in njs]  # xv spans per unit
    tvs = [nsv * SW for nsv in nsvs]

    nc = bacc.Bacc("TRN2", target_bir_lowering=False, debug=False,
                   num_devices=NCORES)

    # x inputs arrive span-major from the host ([s, p, kc, n]) so each
    # span chunk is one DMA of 128 contiguous 8KB lines
    xq_d = nc.dram_tensor("xq", [U, T // SW, 128, 8, SW], F16,
                          kind="ExternalInput")
    xv_ds = [nc.dram_tensor(f"xv{u}", [nsvs[u], 128, 8, SW], F16,
                            kind="ExternalInput") for u in range(U)]
    # weights arrive pre-rearranged host-side: one contiguous line per
    # partition so each loads as a single 128-descriptor DMA
    wq_d = nc.dram_tensor("wq", [128, 8, 128], F16, kind="ExternalInput")
    wk_d = nc.dram_tensor("wk", [128, 8, 128], F16, kind="ExternalInput")
    wv_d = nc.dram_tensor("wv", [128, 8, 130], F16, kind="ExternalInput")
    wf_d = nc.dram_tensor("wf", [128, ATT], F16, kind="ExternalInput")
    mask_ds = [nc.dram_tensor(f"mask{u}", [128, njs[u]], F32,
                              kind="ExternalInput") for u in range(U)]
    bq_d = nc.dram_tensor("bqc", [128, 1], F32, kind="ExternalInput")
    bk_d = nc.dram_tensor("bkc", [128, 1], F32, kind="ExternalInput")
    # y partials tile-major ([n, i, p, c]) so each [128,512] tile is one
    # contiguous 128KB DMA write; the host untiles when assembling
    y_d = nc.dram_tensor("y", [U, 2, 16, 128, 512], F16,
                         kind="ExternalOutput")

    with tile.TileContext(nc) as tc:
        from contextlib import ExitStack
        _st = ExitStack()
        if loop_n > 1:
            _st.enter_context(tc.For_i(0, loop_n, 1))
        with _st, tc.tile_pool(name="persist", bufs=1) as persist:
            qT = [persist.tile([128, T], F16, name=f"qT{u}")
                  for u in range(U)]
            kT = [persist.tile([128, tvs[u]], F16, name=f"kT{u}")
                  for u in range(U)]
            vv = [persist.tile([128, njs[u], 130], F16, name=f"v{u}")
                  for u in range(U)]
            ctxT = [persist.tile([128, T], F16, name=f"ctxT{u}")
                    for u in range(U)]
            wq = persist.tile([128, 8, 128], F16)
            wk = persist.tile([128, 8, 128], F16)
            wv = persist.tile([128, 8, 130], F16)
            wf = persist.tile([128, ATT], F16)
            masks = [persist.tile([128, njs[u]], F32, name=f"mask{u}")
                     for u in range(U)]
            bqc = persist.tile([128, 1], F32)
            bkc = persist.tile([128, 1], F32)
            ones_col = persist.tile([1, 64], F16)

            def set_ones(dst):
                nc.scalar.activation(out=dst, in_=dst, func=AF.Identity,
                                     bias=1.0, scale=0.0)

            with (
                tc.tile_pool(name="chunks", bufs=10) as chunks,
                tc.tile_pool(name="expp", bufs=6) as expp,
                tc.tile_pool(name="workp", bufs=4) as workp,
                tc.tile_pool(name="yp", bufs=6) as yp,
                tc.tile_pool(name="rsd", bufs=4, space="DRAM") as rsd,
                tc.tile_pool(name="pctx", bufs=2, space="PSUM") as pctx,
                tc.tile_pool(name="pcy", bufs=2, space="PSUM") as pcy,
            ):
                def dma_unit_x(u, first=False):
                    """Prefetch all xv and xq spans for unit u; return tiles.

                    For the first unit, the critical-path loads (wk + xv
                    span 0 on sync, wq + xq span 0 on scalar) are issued
                    ahead of everything else.
                    """
                    xvt, xqt = [], []
                    if first:
                        nc.sync.dma_start(out=wk, in_=wk_d[:, :, :])
                        nc.scalar.dma_start(out=wq, in_=wq_d[:, :, :])
                    for s in range(nsvs[u]):
                        c = chunks.tile([128, 8, SW], F16, tag="xc")
                        nc.sync.dma_start(out=c, in_=xv_ds[u][s, :, :, :])
                        xvt.append(c)
                        if first and s == 0:
                            nc.sync.dma_start(out=wv, in_=wv_d[:, :, :])
                    for s in range(T // SW):
                        c = chunks.tile([128, 8, SW], F16, tag="xc")
                        # separate queue from xv so both input streams
                        # move on parallel DMA engines (A0 is DMA-bound)
                        nc.gpsimd.dma_start(out=c, in_=xq_d[u, s, :, :, :])
                        xqt.append(c)
                        if first and s == 0:
                            nc.scalar.dma_start(out=bqc, in_=bq_d[:, :])
                            nc.scalar.dma_start(out=bkc, in_=bk_d[:, :])
                            for uu in range(U):
                                nc.scalar.dma_start(out=masks[uu],
                                                    in_=mask_ds[uu][:, :])
                    if first:
                        nc.scalar.dma_start(out=wf, in_=wf_d[:, :])
                        # memset, not set_ones: activation would read the
                        # uninitialized tile (0*NaN poisons the bias path)
                        nc.vector.memset(ones_col[:, :], 1.0)
                    return xvt, xqt

                def emit_A(u, xvt, xqt, ppq, ppv, pending=None,
                           deferred=None):
                    """Projections for unit u from prefetched chunks."""
                    nj, nsv = njs[u], nsvs[u]
                    # previous unit's final normalization runs here, with
                    # DVE quiet, so its ctx banks are free well before
                    # the next B phase starts
                    if deferred:
                        deferred.pop(0)()
                    for s in range(max(nsv, T // SW)):
                        # C pops go AFTER this span's matmuls (emitted at
                        # the end of the loop body): popping them here
                        # would fill the PE bypass window with tiles
                        # blocked on the just-emitted norm, stalling the
                        # ready projection matmuls behind them
                        # qT columns first: xq rides its own DMA queue
                        # and lands before the sync queue's wk+wv+xv chain,
                        # so q-projection starts the PE earliest
                        xq_c = xqt[s]
                        ps = ppq.tile([128, SW], F32, tag="qk")
                        for kc in range(8):
                            nc.tensor.matmul(
                                ps[:, :], lhsT=wq[:, kc, :],
                                rhs=xq_c[:, kc, :],
                                start=(kc == 0), stop=(kc == 7))
                        with nc.allow_low_precision(reason="q store"):
                            nc.vector.tensor_scalar_add(
                                qT[u][:, s * SW:(s + 1) * SW],
                                ps[:, :], bqc[:, 0:1])
                        if s < nsv:
                            xv_c = xvt[s]
                            # kT columns for this span
                            ps = ppq.tile([128, SW], F32, tag="qk")
                            for kc in range(8):
                                nc.tensor.matmul(
                                    ps[:, :], lhsT=wk[:, kc, :],
                                    rhs=xv_c[:, kc, :],
                                    start=(kc == 0), stop=(kc == 7))
                            with nc.allow_low_precision(reason="k store"):
                                nc.vector.tensor_scalar_add(
                                    kT[u][:, s * SW:(s + 1) * SW],
                                    ps[:, :], bkc[:, 0:1])
                            # v rows for this span
                            for jt in range(SW // 128):
                                j = s * (SW // 128) + jt
                                if j >= nj:
                                    continue
                                psv = ppv.tile([128, 130], F32, tag="v")
                                for kc in range(8):
                                    nc.tensor.matmul(
                                        psv[:, :],
                                        lhsT=xv_c[:, kc,
                                                  jt * 128:(jt + 1) * 128],
                                        rhs=wv[:, kc, :],
                                        start=(kc == 0), stop=(kc == 7))
                                with nc.allow_low_precision(reason="v store"):
                                    nc.vector.tensor_copy(
                                        out=vv[u][:, j, :], in_=psv[:, :])
                                vj = vv[u][:, j, :].rearrange(
                                    "p (h x) -> p h x", x=65)
                                set_ones(vj[:, :, 64:65])
                        if s > 0 and pending:
                            cmode = "act" if s < 3 else "dve"
                            emit_c_tile(*pending.pop(0), mode=cmode)
                            if pending:
                                emit_c_tile(*pending.pop(0), mode=cmode)

                def emit_c_tile(u, i, n, mode="dve"):
                    # mode: which engine copies psum->sbuf. "act" keeps
                    # DVE clear in A windows (ACT is idle there) so the
                    # deferred norm at the next B start isn't queued
                    # behind y copies; "alt" doubles tail throughput.
                    y_ps = pcy.tile([128, 512], F32, tag="cy")
                    nc.tensor.matmul(
                        y_ps[:, :], lhsT=ctxT[u][:, i * 128:(i + 1) * 128],
                        rhs=wf[:, n * 512:(n + 1) * 512],
                        start=True, stop=True)
                    y_sb = yp.tile([128, 512], F16, tag="ysb")
                    use_act = mode == "act" or (mode == "alt"
                                                and (i + n) % 2 == 0)
                    with nc.allow_low_precision(reason="y store"):
                        if use_act:
                            nc.scalar.activation(
                                out=y_sb[:, :], in_=y_ps[:, :],
                                func=AF.Copy, bias=0.0, scale=1.0)
                        else:
                            nc.vector.tensor_copy(out=y_sb[:, :],
                                                  in_=y_ps[:, :])
                    nc.gpsimd.dma_start(out=y_d[u, n, i, :, :],
                                        in_=y_sb[:, :])

                def make_norm(u, ctx_ps, ibs, ib, pending):
                    def norm():
                        for hh in range(2):
                            p0 = hh * 64
                            rs = workp.tile([1, 512], F16, tag="rs")
                            with nc.allow_low_precision(reason="recip"):
                                nc.vector.reciprocal(
                                    out=rs[:, :], in_=ctx_ps[hh][64:65, :])
                            # unnormalized ctx to SBUF (frees the psum
                            # bank), then scale in place by the PE-
                            # broadcast reciprocal row (single PSUM input)
                            with nc.allow_low_precision(reason="ctx store"):
                                nc.vector.tensor_copy(
                                    out=ctxT[u][p0:p0 + 64, ibs],
                                    in_=ctx_ps[hh][0:64, :])
                            bc = pcy.tile([128, 512], F32, tag="cy")
                            nc.tensor.matmul(
                                bc[0:64, :], lhsT=ones_col[:, :],
                                rhs=rs[:, :], start=True, stop=True)
                            with nc.allow_low_precision(reason="ctx scale"):
                                nc.vector.tensor_mul(
                                    ctxT[u][p0:p0 + 64, ibs],
                                    ctxT[u][p0:p0 + 64, ibs], bc[0:64, :])
                        pending.extend(
                            (u, i, n) for i in range(ib * 4, ib * 4 + 4)
                            for n in range(2))
                    return norm

                def emit_B(u, pe_pool, pending, deferred):
                    nj = njs[u]
                    LAG = 3  # ctx trails the energy/exp stream by 3 j's

                    def emit_ctx(ctx_ps, ex, j):
                        for hh in range(2):
                            nc.tensor.matmul(
                                ctx_ps[hh],
                                lhsT=vv[u][:, j, hh * 65:(hh + 1) * 65],
                                rhs=ex[:, hh * 512:(hh + 1) * 512],
                                start=(j == 0), stop=(j == nj - 1))

                    for ib in range(4):
                        _mark(nc, f"B{u}.{ib}")
                        ibs = slice(ib * 512, (ib + 1) * 512)
                        ctxA = pctx.tile([65, 512], F32, tag="ctx")
                        ctxB = pctx.tile([65, 512], F32, tag="ctx")
                        ctx_ps = (ctxA[:, :], ctxB[:, :])
                        lagq = []
                        for j in range(nj):
                            e_ps = pe_pool.tile([128, 1024], F32, tag="e")
                            for hh in range(2):
                                p0 = hh * 64
                                nc.tensor.matmul(
                                    e_ps[:, hh * 512:(hh + 1) * 512],
                                    lhsT=kT[u][p0:p0 + 64,
                                               j * 128:(j + 1) * 128],
                                    rhs=qT[u][p0:p0 + 64, ibs],
                                    start=True, stop=True)
                            # previous ib's normalization slots in here:
                            # its ctx banks are not needed until this
                            # ib's first ctx matmul below
                            if j == 0 and deferred:
                                deferred.pop(0)()
                            ex = expp.tile([128, 1024], F16, tag="ex")
                            nc.scalar.activation(
                                out=ex[:, :], in_=e_ps[:, :], func=AF.Exp,
                                bias=masks[u][:, j:j + 1], scale=1.0)
                            lagq.append((ex, j))
                            if pending and (j % 2 == 1 or len(pending) > 8):
                                emit_c_tile(*pending.pop(0))
                            # consume the exp from LAG iterations ago: PE
                            # never waits on a fresh exp, taking the
                            # e->exp->ctx semaphore latency off the
                            # per-iteration critical cycle
                            if len(lagq) > LAG:
                                exl, jl = lagq.pop(0)
                                emit_ctx(ctx_ps, exl, jl)
                        while lagq:
                            exl, jl = lagq.pop(0)
                            emit_ctx(ctx_ps, exl, jl)
                            if len(pending) > 4:
                                emit_c_tile(*pending.pop(0))
                        deferred.append(
                            make_norm(u, ctx_ps, ibs, ib,
                                      pending if "C" in parts else []))
                    # final ib's norm drains at the start of the next
                    # unit's A phase, or after the loop for the last unit

                # ---- schedule ----
                REGIONS.clear()
                pending = []
                deferred = []
                _mark(nc, "init")
                xnext = dma_unit_x(0, first=True)
                for u in range(U):
                    xvt, xqt = xnext
                    _mark(nc, f"A{u}")
                    with (
                        tc.tile_pool(name=f"ppq{u}", bufs=2,
                                     space="PSUM") as ppq,
                        tc.tile_pool(name=f"ppv{u}", bufs=2,
                                     space="PSUM") as ppv,
                    ):
                        emit_A(u, xvt, xqt, ppq, ppv, pending)
                    if u + 1 < U:
                        xnext = dma_unit_x(u + 1)
                    if "B" in parts:
                        with tc.tile_pool(name=f"pe{u}", bufs=2,
                                          space="PSUM") as pe_pool:
                            emit_B(u, pe_pool, pending, deferred)
                _mark(nc, "tail")
                while deferred:
                    deferred.pop(0)()
                while pending:
                    emit_c_tile(*pending.pop(0), mode="alt")
                _mark(nc, "end")
    nc.compile()
    _cache[key] = nc
    return nc


def make_in_maps(query, value, value_lens, Wq, bq, Wk, bk, Wv, bv, Wf, bf,
                 mm_np=np.float16):
    query = np.ascontiguousarray(np.asarray(query, np.float32))
    value = np.ascontiguousarray(np.asarray(value, np.float32))
    value_lens = np.asarray(value_lens)
    Wq = np.asarray(Wq, np.float32)
    Wk = np.asarray(Wk, np.float32)
    Wv = np.asarray(Wv, np.float32)
    Wf = np.asarray(Wf, np.float32)
    bq = np.asarray(bq, np.float32)
    bk = np.asarray(bk, np.float32)

    scale = 1.0 / np.sqrt(np.float32(DH))
    effL = [int(l) if l > 0 else T for l in value_lens]
    njs_b = [max(1, int(np.ceil(L / 128))) for L in effL]
    # ascending NJ: the longest unit runs last, so its many ACT-bound
    # j-iterations absorb the final-projection backlog before the tail
    bord = sorted(range(B), key=lambda b: njs_b[b])
    njs = tuple(njs_b[b] for b in bord)
    nsvs = [(nj * 128 + SW - 1) // SW for nj in njs]
    tvs = [nsv * SW for nsv in nsvs]

    # per-unit shared arrays (identical across cores), packed span-major
    # [s, p, kc, n] so each on-device chunk DMA is fully contiguous
    def pack_x(xT, tv):  # xT: [D, T'] -> [nsv, 128, 8, SW]
        nsv = tv // SW
        return np.ascontiguousarray(
            xT[:, :tv].reshape(8, 128, nsv, SW).transpose(2, 1, 0, 3))

    xq_all = np.empty((len(bord), T // SW, 128, 8, SW), mm_np)
    for u, b in enumerate(bord):
        if value_lens[b] == 0:
            xq_all[u] = 0
        else:
            xq_all[u] = pack_x(query[b].T.astype(mm_np), T)
    xv_us = [pack_x(value[b].T.astype(mm_np), tvs[u])
             for u, b in enumerate(bord)]
    mask_us = []
    for u, b in enumerate(bord):
        nj = njs[u]
        mask = np.zeros((128, nj), np.float32)
        L = int(value_lens[b])
        if L > 0:
            idx = np.arange(nj * 128).reshape(nj, 128).T  # [128, nj]
            mask[idx >= L] = -LARGE
        mask_us.append(mask)

    def warr(w):  # [D, M] -> [128, 8, M] partition-major for 1-line DMA
        return np.ascontiguousarray(
            w.reshape(8, 128, -1).transpose(1, 0, 2)).astype(mm_np)

    in_maps = []
    for c in range(NCORES):
        cs = slice(c * 128, (c + 1) * 128)
        wv_c = np.zeros((D, 130), np.float32)
        wv_c[:, 0:64] = Wv[:, c * 128:c * 128 + 64]
        wv_c[:, 65:129] = Wv[:, c * 128 + 64:c * 128 + 128]
        m = {
            "xq": xq_all,
            "wq": warr(Wq[:, cs] * scale),
            "wk": warr(Wk[:, cs]),
            "wv": warr(wv_c),
            "wf": Wf[cs, :].astype(mm_np),
            "bqc": (bq[cs] * scale).reshape(128, 1).copy(),
            "bkc": bk[cs].reshape(128, 1).copy(),
        }
        for u in range(len(bord)):
            m[f"xv{u}"] = xv_us[u]
            m[f"mask{u}"] = mask_us[u]
        in_maps.append(m)
    return in_maps, njs


def assemble(results, value_lens, Wv, bv, Wf, bf):
    value_lens = np.asarray(value_lens)
    bv = np.asarray(bv, np.float32)
    Wf = np.asarray(Wf, np.float32)
    bf = np.asarray(bf, np.float32)
    effL = [int(l) if l > 0 else T for l in value_lens]
    njs_b = [max(1, int(np.ceil(L / 128))) for L in effL]
    bord = sorted(range(B), key=lambda b: njs_b[b])
    const = (bv @ Wf + bf).astype(np.float32)
    out = np.empty((B, T, ATT), np.float32)
    for u, b in enumerate(bord):
        acc = np.zeros((2, 16, 128, 512), np.float32)
        for c in range(NCORES):
            acc += results[c]["y"][u]
        # untile [n, i, p, c] -> [i*128+p, n*512+c]
        out[b] = acc.transpose(1, 2, 0, 3).reshape(T, ATT) + const
    return out


def kernel(query, value, value_lens, Wq, bq, Wk, bk, Wv, bv, Wf, bf):
    in_maps, njs = make_in_maps(query, value, value_lens, Wq, bq, Wk, bk,
                                Wv, bv, Wf, bf)
    nc = build_nc(njs)
    res = run_bass_kernel_spmd(nc, in_maps, list(range(NCORES)))
    return assemble(res.results, value_lens, Wv, bv, Wf, bf)
